# revision 5
# baseline (speedup 1.0000x reference)
"""Trainium2 Bass kernel for nn_BitwiseMLP: 3x (Linear + training-mode BatchNorm).

Math: reference computes, per layer,  h = gamma * (y - mean_B(y)) * rsqrt(var_B(y) + eps) + beta
with y = x @ W.T + b.  BatchNorm is invariant to per-feature constant shifts of y, so
  - every linear bias b_l cancels exactly,
  - the additive part of each BN affine (beta_l - a_l*mean_l) feeds the next linear as a
    per-feature constant -> also cancels under the next BN.
Only the multiplicative scales a_l = gamma_l * rsqrt(var_l + eps) propagate (folded into the
next layer's input activations), plus one final affine a2*u2 + (beta2 - a2*mean2) on the output.

Device layout: everything transposed -> activations are [features, batch_rows] so BN stats are
free-axis reductions and scales are per-partition multiplies. Batch is sharded 8 ways
(2048 rows/core); weights replicated. Matmuls in bf16 (fp32 PSUM accumulate), stats fp32,
cross-core stats via small AllReduces (3 per layer, chunked so they pipeline on the CC stream).

Schedule (PE-idle elimination):
  - L0 opens with strips 0,1 interleaved j-outer so the PE consumes each arriving
    xt_j/w0_j DMA pair at 2 strips' worth of matmuls (input load is HBM-bound).
  - L1/L2 open with a 4-strip staged split: strips 0..3 run k<12 first (~50us of PE
    work that needs only the previous layer's first two stat chunks), so the last
    chunk's AllReduce + cross-core skew is fully hidden. Strips 0,1 park their
    k<12 partial in SBUF (bf16) to free PSUM banks for strips 2,3, and finish with
    a DVE add; strips 2,3 simply keep their PSUM banks open across the split.
  - Stats pipeline (cc_in DMA -> collective doorbell -> result readback) lives on
    the gpsimd queue; weight prefetch + output writeback live on the sync queue,
    so a doorbell waiting on a semaphore never delays bulk transfers.
  - L2 writes bf16 output per strip as soon as its chunk's affine lands; chunks
    [0,4,6,7,8] make the final exposed chunk a single strip.
"""

import numpy as np
import ml_dtypes

# ---- problem constants (full size; hardcoded per harness contract) ----
N_CORES = 8
B_FULL = 16384
D_IN = 1024
D_H = 2048
D_OUT = 1024
BN_EPS = 1e-5

_PROG_CACHE = {}
LAST_RESULTS = None  # BassKernelResults of the most recent run (for test harness)


def build_program(R, B_total):
    """Build the per-core Bass program. R = batch rows per core (multiple of 512)."""
    import concourse.bacc as bacc
    import concourse.mybir as mybir
    import concourse.tile as tile

    f32 = mybir.dt.float32
    bf16 = mybir.dt.bfloat16
    Alu = mybir.AluOpType
    Act = mybir.ActivationFunctionType

    NT = R // 512  # n-chunks of 512 rows
    assert R % 512 == 0
    KT = [D_IN // 128, D_H // 128, D_H // 128]  # k-tiles per layer
    MT = [D_H // 128, D_H // 128, D_OUT // 128]  # m-strips per layer
    inv_B = 1.0 / float(B_total)
    GROUP = [list(range(N_CORES))]

    full_size = MT[0] >= 16 and NT >= 4

    nc = bacc.Bacc(None, num_devices=N_CORES)

    xt_d = nc.dram_tensor("xt", [D_IN, R], bf16, kind="ExternalInput")
    w0_d = nc.dram_tensor("w0t", [D_IN, D_H], bf16, kind="ExternalInput")
    # w1/w2 pre-tiled on host: [m_strip, partition(k%128), k//128 * 128 + f]
    # so each strip DMA is one [128, KT*128] transfer with 4KB contiguous lines.
    w1_d = nc.dram_tensor("w1t", [MT[1], 128, KT[1] * 128], bf16, kind="ExternalInput")
    w2_d = nc.dram_tensor("w2t", [MT[2], 128, KT[2] * 128], bf16, kind="ExternalInput")
    g0_d = nc.dram_tensor("g0", [D_H], f32, kind="ExternalInput")
    g1_d = nc.dram_tensor("g1", [D_H], f32, kind="ExternalInput")
    g2_d = nc.dram_tensor("g2", [D_OUT], f32, kind="ExternalInput")
    b2_d = nc.dram_tensor("beta2", [D_OUT], f32, kind="ExternalInput")
    out_d = nc.dram_tensor("out", [D_OUT, R], bf16, kind="ExternalOutput")

    # stats chunking: [0,8,12,16] issues collectives early enough that each is
    # done (or nearly) by the time its scales are consumed; L2's chunks are
    # grouped by strip completion order under the staged start (2,3,0,1,4..7)
    # with single-strip last chunks to minimize the exposed tail.
    if full_size:
        CHB = [[0, 8, 12, 16], [0, 8, 12, 16], [0, 4, 6, 7, 8]]
    else:  # small sim shapes
        CHB = [[0, MT[0] // 2, MT[0]], [0, MT[1] // 2, MT[1]], [0, MT[2] // 2, MT[2]]]
    cc_in = [
        [
            nc.dram_tensor(f"cc_in{l}_{q}", [128, 2 * (b - a)], f32)
            for q, (a, b) in enumerate(zip(CHB[l], CHB[l][1:]))
        ]
        for l in range(3)
    ]
    cc_out = [
        [
            nc.dram_tensor(
                f"cc_out{l}_{q}", [128, 2 * (b - a)], f32, addr_space="Shared"
            )
            for q, (a, b) in enumerate(zip(CHB[l], CHB[l][1:]))
        ]
        for l in range(3)
    ]

    with tile.TileContext(nc) as tc:
        import contextlib

        with contextlib.ExitStack() as ctx:
            # one slot size (4KB/partition) for all activation/weight strips;
            # ring reuse: xt+w0 (16) -> u0 (16) -> u1 (reuses xt/w0) -> u2 (reuses u0)
            act = ctx.enter_context(tc.tile_pool(name="act", bufs=32))
            wpool = ctx.enter_context(tc.tile_pool(name="wstrip", bufs=6))
            pspool = ctx.enter_context(tc.tile_pool(name="psum", bufs=8, space="PSUM"))
            small = ctx.enter_context(tc.tile_pool(name="small", bufs=1))

            # ---- resident loads first; j=0 strips split into 512-col chunks so
            # the first matmul can start ~4us earlier ----
            xt_r = xt_d[:].rearrange("(j p) r -> p j r", p=128)
            w0_r = w0_d[:].rearrange("(j p) f -> p j f", p=128)
            xts, w0s = [], []
            for j in range(KT[0]):
                wt = act.tile([128, D_H], bf16, tag="act", name=f"w0_{j}")
                if j == 0 and full_size:
                    for c in range(4):
                        nc.sync.dma_start(
                            out=wt[:, c * 512 : (c + 1) * 512],
                            in_=w0_r[:, j, c * 512 : (c + 1) * 512],
                        )
                else:
                    nc.sync.dma_start(out=wt, in_=w0_r[:, j, :])
                w0s.append(wt)
                xtile = act.tile([128, R], bf16, tag="act", name=f"xt_{j}")
                if j == 0 and full_size:
                    for c in range(NT):
                        nc.gpsimd.dma_start(
                            out=xtile[:, c * 512 : (c + 1) * 512],
                            in_=xt_r[:, j, c * 512 : (c + 1) * 512],
                        )
                else:
                    nc.gpsimd.dma_start(out=xtile, in_=xt_r[:, j, :])
                xts.append(xtile)

            # ---- constants / per-feature params ----
            eps_t = small.tile([128, 1], f32, tag="eps")
            nc.vector.memset(eps_t, BN_EPS)
            g_t = []
            for l, gd in enumerate((g0_d, g1_d, g2_d)):
                t = small.tile([128, MT[l]], f32, tag=f"g{l}", name=f"g{l}")
                nc.sync.dma_start(out=t, in_=gd[:].rearrange("(m p) -> p m", p=128))
                g_t.append(t)
            b2_t = small.tile([128, MT[2]], f32, tag="b2")
            nc.sync.dma_start(out=b2_t, in_=b2_d[:].rearrange("(m p) -> p m", p=128))

            def u_strips(pool_tag, count, dtype, cols):
                return [
                    act.tile([128, cols], dtype, tag="act", name=f"{pool_tag}_{j}")
                    for j in range(count)
                ]

            def stats_half(l, BN, h, want_c, beta_t, sg_eng=None):
                """bn_stats partials (feature chunk h) -> S/Q -> allreduce -> a [, c].

                cc_in DMA + collective doorbell sit back-to-back on the gpsimd
                queue so every doorbell fires as soon as its local stats land —
                the CC stream then runs ops the moment it frees up. The result
                readback (sg) goes on sg_eng (default gpsimd; L2 passes sync so
                a readback waiting on a slow AllReduce never delays the NEXT
                chunk's doorbell in the gpsimd FIFO).
                """
                m0, m1 = CHB[l][h], CHB[l][h + 1]
                mh = m1 - m0
                mv = small.tile([128, mh, 2], f32, tag=f"mv{l}{h}", name=f"mv{l}{h}")
                for m in range(m0, m0 + mh):
                    nc.vector.bn_aggr(
                        out=mv[:, m - m0, :],
                        in_=BN[:, m * NT * 6 : (m + 1) * NT * 6],
                    )
                # S = mean*R ; Q = (var + mean^2)*R  (exact cross-core sums)
                sf = small.tile([128, 2, mh], f32, tag=f"sf{l}{h}", name=f"sf{l}{h}")
                nc.vector.tensor_scalar_mul(sf[:, 0, :], mv[:, :, 0], float(R))
                nc.vector.tensor_mul(sf[:, 1, :], mv[:, :, 0], mv[:, :, 0])
                nc.vector.tensor_add(sf[:, 1, :], sf[:, 1, :], mv[:, :, 1])
                nc.vector.tensor_scalar_mul(sf[:, 1, :], sf[:, 1, :], float(R))
                nc.gpsimd.dma_start(out=cc_in[l][h][:], in_=sf)
                nc.gpsimd.collective_compute(
                    "AllReduce",
                    Alu.add,
                    replica_groups=GROUP,
                    ins=[cc_in[l][h][:]],
                    outs=[cc_out[l][h][:]],
                )
                sg = small.tile([128, 2, mh], f32, tag=f"sg{l}{h}", name=f"sg{l}{h}")
                (sg_eng or nc.gpsimd).dma_start(
                    out=sg, in_=cc_out[l][h][:].rearrange("p (s m) -> p s m", s=2)
                )
                mean = small.tile([128, mh], f32, tag=f"mean{l}{h}", name=f"mean{l}{h}")
                var = small.tile([128, mh], f32, tag=f"var{l}{h}", name=f"var{l}{h}")
                tmp = small.tile([128, mh], f32, tag=f"tmp{l}{h}", name=f"tmp{l}{h}")
                nc.vector.tensor_scalar_mul(mean, sg[:, 0, :], inv_B)
                nc.vector.tensor_scalar_mul(var, sg[:, 1, :], inv_B)
                nc.vector.tensor_mul(tmp, mean, mean)
                nc.vector.tensor_sub(var, var, tmp)
                # var <- sqrt(var + eps), then reciprocal -> rstd
                nc.scalar.activation(out=var, in_=var, func=Act.Sqrt, bias=eps_t[:, 0:1])
                nc.vector.reciprocal(out=var, in_=var)
                a = small.tile([128, mh], f32, tag=f"a{l}{h}", name=f"a{l}{h}")
                nc.vector.tensor_mul(a, var, g_t[l][:, m0 : m0 + mh])
                if not want_c:
                    return a, None
                c = small.tile([128, mh], f32, tag=f"c{l}{h}", name=f"c{l}{h}")
                nc.vector.tensor_mul(tmp, a, mean)
                nc.vector.tensor_sub(c, beta_t[:, m0 : m0 + mh], tmp)
                return a, c

            SPLIT = 12  # k-tile where the staged strips pause (chunk boundary)

            def layer(l, lhs_getter, rhs_at, dest_at, finish_chunk=None):
                """One linear layer with a staged start and chunked stats.

                Emission order of strips:
                  l=0:        [pair(0,1) j-outer], 2, 3, ..., MT-1
                  l=1,2:      0A,1A,2A,3A, 2B,3B,0B,1B, 4, 5, ..., MT-1
                finish_chunk(q, BN) is emitted as soon as every strip of chunk q
                has its bn_stats emitted.
                """
                BN = small.tile([128, MT[l] * NT * 6], f32, tag=f"BN{l}", name=f"BN{l}")
                done = set()
                ch = [0]

                def strip_stats(m):
                    done.add(m)
                    while (
                        finish_chunk is not None
                        and ch[0] < len(CHB[l]) - 1
                        and all(s in done for s in range(CHB[l][ch[0]], CHB[l][ch[0] + 1]))
                    ):
                        finish_chunk(ch[0], BN)
                        ch[0] += 1

                def psum_copy_stats(m, pss):
                    for n in range(NT):
                        idx = m * NT + n
                        nc.scalar.activation(
                            out=dest_at(m, n), in_=pss[n], func=Act.Copy
                        )
                        nc.vector.bn_stats(
                            out=BN[:, idx * 6 : idx * 6 + 6], in_=pss[n]
                        )
                    strip_stats(m)

                def alloc_ps(m):
                    return [
                        pspool.tile([128, 512], f32, tag="ps", name=f"ps{l}_{m}_{n}")
                        for n in range(NT)
                    ]

                def mm_block(pss, lhs, j, start, stop):
                    for n in range(NT):
                        nc.tensor.matmul(
                            pss[n], lhs(j), rhs_at(j, n), start=start, stop=stop
                        )

                def full_strip(m):
                    lhs = lhs_getter(m)
                    pss = alloc_ps(m)
                    for j in range(KT[l]):
                        mm_block(pss, lhs, j, j == 0, j == KT[l] - 1)
                    psum_copy_stats(m, pss)

                if not full_size:
                    for m in range(MT[l]):
                        full_strip(m)
                    return BN

                if l == 0:
                    # pair-interleave strips 0,1 j-outer: consume each arriving
                    # xt_j/w0_j DMA with two strips' worth of matmuls.
                    lhs0, lhs1 = lhs_getter(0), lhs_getter(1)
                    ps0, ps1 = alloc_ps(0), alloc_ps(1)
                    for j in range(KT[0]):
                        mm_block(ps0, lhs0, j, j == 0, j == KT[0] - 1)
                        mm_block(ps1, lhs1, j, j == 0, j == KT[0] - 1)
                    psum_copy_stats(0, ps0)
                    psum_copy_stats(1, ps1)
                    for m in range(2, MT[l]):
                        full_strip(m)
                    return BN

                # l = 1, 2: staged split. Strips 0..3 run k<SPLIT before anything
                # needs the previous layer's last stat chunk (k>=SPLIT scales).
                lhs = [lhs_getter(m) for m in range(4)]
                ps = {}
                # 0A, 1A: k<SPLIT, close PSUM, park partial (bf16) in dest strips
                for m in (0, 1):
                    ps[m] = alloc_ps(m)
                    for j in range(SPLIT):
                        mm_block(ps[m], lhs[m], j, j == 0, j == SPLIT - 1)
                    for n in range(NT):
                        nc.scalar.activation(
                            out=dest_at(m, n), in_=ps[m][n], func=Act.Copy
                        )
                # 2A, 3A: k<SPLIT, keep PSUM banks open across the split
                for m in (2, 3):
                    ps[m] = alloc_ps(m)
                    for j in range(SPLIT):
                        mm_block(ps[m], lhs[m], j, j == 0, False)
                # 2B, 3B: finish accumulation in the still-open banks
                for m in (2, 3):
                    for j in range(SPLIT, KT[l]):
                        mm_block(ps[m], lhs[m], j, False, j == KT[l] - 1)
                    psum_copy_stats(m, ps[m])
                # 0B, 1B: fresh PSUM group for k>=SPLIT, then DVE add onto the
                # parked partial; bn_stats read the (bf16) summed strip.
                for m in (0, 1):
                    psb = alloc_ps(m)
                    for j in range(SPLIT, KT[l]):
                        mm_block(psb, lhs[m], j, j == SPLIT, j == KT[l] - 1)
                    for n in range(NT):
                        idx = m * NT + n
                        d = dest_at(m, n)
                        nc.vector.tensor_add(d, psb[n], d)
                        nc.vector.bn_stats(out=BN[:, idx * 6 : idx * 6 + 6], in_=d)
                    strip_stats(m)
                for m in range(4, MT[l]):
                    full_strip(m)
                return BN

            def strips_rhs(strips):
                return lambda j, n: strips[j][:, n * 512 : (n + 1) * 512]

            def scale_one(strips, j, ac):
                s = strips[j][:]
                if j % 4 == 3:
                    nc.scalar.activation(out=s, in_=s, func=Act.Copy, scale=ac)
                else:
                    nc.vector.tensor_scalar_mul(s, s, ac)

            def finisher(l, u_next):
                def fin(q, BN):
                    a, _ = stats_half(l, BN, q, False, None)
                    m0 = CHB[l][q]
                    for j in range(m0, CHB[l][q + 1]):
                        scale_one(u_next, j, a[:, j - m0 : j - m0 + 1])
                return fin

            # ================= layer 0 =================
            u0 = u_strips("u0", MT[0], bf16, R)

            def lhs0(m):
                return lambda j: w0s[j][:, m * 128 : (m + 1) * 128]

            layer(0, lhs0, strips_rhs(xts), lambda m, n: strips_rhs(u0)(m, n),
                  finisher(0, u0))

            # ================= layer 1 =================
            u1 = u_strips("u1", MT[1], bf16, R)

            def lhs_strip(w_dram, l):
                def getter(m):
                    w = wpool.tile([128, KT[l] * 128], bf16, tag="w", name=f"w{l}_{m}")
                    nc.sync.dma_start(out=w, in_=w_dram[m])
                    return lambda j: w[:, j * 128 : (j + 1) * 128]

                return getter

            layer(1, lhs_strip(w1_d, 1), strips_rhs(u0), strips_rhs(u1),
                  finisher(1, u1))

            # ================= layer 2 =================
            # u2: bf16 strips, affine in place per chunk, immediate bf16 writeback
            u2 = u_strips("u2", MT[2], bf16, R)

            def fin2(q, BN):
                last_q = q == len(CHB[2]) - 2
                # earlier chunks: sg on sync (never blocks the next doorbell);
                # last chunk: sg on gpsimd (free after the final doorbell, while
                # sync may still be draining a writeback).
                a, c = stats_half(
                    2, BN, q, True, b2_t,
                    sg_eng=nc.gpsimd if last_q else nc.sync,
                )
                m0 = CHB[2][q]
                for m in range(m0, CHB[2][q + 1]):
                    am = a[:, m - m0 : m - m0 + 1]
                    cm = c[:, m - m0 : m - m0 + 1]
                    s = u2[m][:]
                    if m % 2 == 0:
                        nc.vector.tensor_scalar(s, s, am, cm, Alu.mult, Alu.add)
                    else:
                        nc.scalar.activation(
                            out=s, in_=s, func=Act.Identity, bias=cm, scale=am
                        )
                    if last_q and m == CHB[2][q + 1] - 1:
                        # final strip: halve the exposed writeback across queues
                        H = R // 2
                        nc.sync.dma_start(
                            out=out_d[m * 128 : (m + 1) * 128, 0:H], in_=u2[m][:, 0:H]
                        )
                        nc.gpsimd.dma_start(
                            out=out_d[m * 128 : (m + 1) * 128, H:R], in_=u2[m][:, H:R]
                        )
                    else:
                        nc.sync.dma_start(
                            out=out_d[m * 128 : (m + 1) * 128, :], in_=s
                        )

            layer(2, lhs_strip(w2_d, 2), strips_rhs(u1), strips_rhs(u2), fin2)

    nc.compile()
    return nc


def _get_program(R, B_total):
    key = (R, B_total)
    if key not in _PROG_CACHE:
        _PROG_CACHE[key] = build_program(R, B_total)
    return _PROG_CACHE[key]


def prep_inputs(x, W0, W1, W2, gamma0, gamma1, gamma2, beta2, n_cores=N_CORES):
    """Host-side: transpose, cast to bf16, shard batch columns."""
    bf = ml_dtypes.bfloat16

    def strip_tiles(W):
        # W [F, K] -> [F//128 strips, 128 partitions(k%128), (K//128)*128] bf16
        # element [m, p, j*128+f] = W[m*128+f, j*128+p]
        F, Kd = W.shape
        wt = W.T.reshape(Kd // 128, 128, F // 128, 128)  # [j, p, m, f]
        return np.ascontiguousarray(wt.transpose(2, 1, 0, 3)).reshape(
            F // 128, 128, Kd // 128 * 128
        ).astype(bf)

    xT = np.ascontiguousarray(x.T)  # [D_IN, B]
    R = x.shape[0] // n_cores
    w0t = np.ascontiguousarray(W0.T).astype(bf)
    w1t = strip_tiles(np.asarray(W1, dtype=np.float32))
    w2t = strip_tiles(np.asarray(W2, dtype=np.float32))
    g0 = np.ascontiguousarray(gamma0, dtype=np.float32)
    g1 = np.ascontiguousarray(gamma1, dtype=np.float32)
    g2 = np.ascontiguousarray(gamma2, dtype=np.float32)
    b2 = np.ascontiguousarray(beta2, dtype=np.float32)
    in_maps = []
    for c in range(n_cores):
        in_maps.append(
            {
                "xt": np.ascontiguousarray(xT[:, c * R : (c + 1) * R]).astype(bf),
                "w0t": w0t,
                "w1t": w1t,
                "w2t": w2t,
                "g0": g0,
                "g1": g1,
                "g2": g2,
                "beta2": b2,
            }
        )
    return in_maps, R


def kernel(
    x,
    W0,
    b0,
    gamma0,
    beta0,
    W1,
    b1,
    gamma1,
    beta1,
    W2,
    b2,
    gamma2,
    beta2,
):
    """Full-input entry point: shard across 8 NeuronCores, run, gather.

    b0/b1/b2/beta0/beta1 cancel exactly under training-mode BatchNorm
    (shift invariance), so they are not shipped to the device.
    """
    global LAST_RESULTS
    from concourse.bass_utils import run_bass_kernel_spmd

    x = np.asarray(x, dtype=np.float32)
    B = x.shape[0]
    in_maps, R = prep_inputs(
        x, np.asarray(W0), np.asarray(W1), np.asarray(W2),
        np.asarray(gamma0), np.asarray(gamma1), np.asarray(gamma2),
        np.asarray(beta2),
    )
    nc = _get_program(R, B)
    res = run_bass_kernel_spmd(nc, in_maps, core_ids=list(range(N_CORES)))
    LAST_RESULTS = res
    out = np.empty((B, D_OUT), dtype=np.float32)
    for c in range(N_CORES):
        out[c * R : (c + 1) * R, :] = res.results[c]["out"].T.astype(np.float32)
    return out


# revision 6
# speedup vs baseline: 1.2048x; 1.2048x over previous
"""Trainium2 Bass kernel for nn_BitwiseMLP: 3x (Linear + training-mode BatchNorm).

Math: reference computes, per layer,  h = gamma * (y - mean_B(y)) * rsqrt(var_B(y) + eps) + beta
with y = x @ W.T + b.  BatchNorm is invariant to per-feature constant shifts of y, so
  - every linear bias b_l cancels exactly,
  - the additive part of each BN affine (beta_l - a_l*mean_l) feeds the next linear as a
    per-feature constant -> also cancels under the next BN.
Only the multiplicative scales a_l = gamma_l * rsqrt(var_l + eps) propagate (folded into the
next layer's input activations), plus one final affine a2*u2 + (beta2 - a2*mean2) on the output.

Device layout: everything transposed -> activations are [features, batch_rows] so BN stats are
free-axis reductions and scales are per-partition multiplies. Batch is sharded 8 ways
(2048 rows/core); weights replicated. Matmuls in bf16 (fp32 PSUM accumulate), stats fp32,
cross-core stats via small AllReduces (3 per layer, chunked so they pipeline on the CC stream).

Schedule (PE-idle elimination):
  - L0 opens with strips 0,1 interleaved j-outer so the PE consumes each arriving
    xt_j/w0_j DMA pair at 2 strips' worth of matmuls (input load is HBM-bound).
  - L1/L2 open with a 4-strip staged split: strips 0..3 run k<12 first (~50us of PE
    work that needs only the previous layer's first two stat chunks), so the last
    chunk's AllReduce + cross-core skew is fully hidden. Strips 0,1 park their
    k<12 partial in SBUF (bf16) to free PSUM banks for strips 2,3, and finish with
    a DVE add; strips 2,3 simply keep their PSUM banks open across the split.
  - Stats pipeline (cc_in DMA -> collective doorbell -> result readback) lives on
    the gpsimd queue; weight prefetch + output writeback live on the sync queue,
    so a doorbell waiting on a semaphore never delays bulk transfers.
  - L2 writes bf16 output per strip as soon as its chunk's affine lands; chunks
    [0,4,6,7,8] make the final exposed chunk a single strip.
"""

import numpy as np
import ml_dtypes

# ---- problem constants (full size; hardcoded per harness contract) ----
N_CORES = 8
B_FULL = 16384
D_IN = 1024
D_H = 2048
D_OUT = 1024
BN_EPS = 1e-5

_PROG_CACHE = {}
LAST_RESULTS = None  # BassKernelResults of the most recent run (for test harness)


def build_program(R, B_total):
    """Build the per-core Bass program. R = batch rows per core (multiple of 512)."""
    import concourse.bacc as bacc
    import concourse.mybir as mybir
    import concourse.tile as tile

    f32 = mybir.dt.float32
    bf16 = mybir.dt.bfloat16
    Alu = mybir.AluOpType
    Act = mybir.ActivationFunctionType

    NT = R // 512  # n-chunks of 512 rows
    assert R % 512 == 0
    KT = [D_IN // 128, D_H // 128, D_H // 128]  # k-tiles per layer
    MT = [D_H // 128, D_H // 128, D_OUT // 128]  # m-strips per layer
    inv_B = 1.0 / float(B_total)
    GROUP = [list(range(N_CORES))]

    full_size = MT[0] >= 16 and NT >= 4

    nc = bacc.Bacc(None, num_devices=N_CORES)

    xt_d = nc.dram_tensor("xt", [D_IN, R], bf16, kind="ExternalInput")
    w0_d = nc.dram_tensor("w0t", [D_IN, D_H], bf16, kind="ExternalInput")
    # w1/w2 pre-tiled on host: [m_strip, partition(k%128), k//128 * 128 + f]
    # so each strip DMA is one [128, KT*128] transfer with 4KB contiguous lines.
    w1_d = nc.dram_tensor("w1t", [MT[1], 128, KT[1] * 128], bf16, kind="ExternalInput")
    w2_d = nc.dram_tensor("w2t", [MT[2], 128, KT[2] * 128], bf16, kind="ExternalInput")
    g0_d = nc.dram_tensor("g0", [D_H], f32, kind="ExternalInput")
    g1_d = nc.dram_tensor("g1", [D_H], f32, kind="ExternalInput")
    g2_d = nc.dram_tensor("g2", [D_OUT], f32, kind="ExternalInput")
    b2_d = nc.dram_tensor("beta2", [D_OUT], f32, kind="ExternalInput")
    out_d = nc.dram_tensor("out", [D_OUT, R], bf16, kind="ExternalOutput")

    # stats chunking: [0,8,12,16] issues collectives early enough that each is
    # done (or nearly) by the time its scales are consumed; L2's chunks are
    # grouped by strip completion order under the staged start (2,3,0,1,4..7)
    # with single-strip last chunks to minimize the exposed tail.
    if full_size:
        CHB = [[0, 8, 12, 16], [0, 8, 12, 16], [0, 4, 6, 8]]
    else:  # small sim shapes
        CHB = [[0, MT[0] // 2, MT[0]], [0, MT[1] // 2, MT[1]], [0, MT[2] // 2, MT[2]]]
    cc_in = [
        [
            nc.dram_tensor(f"cc_in{l}_{q}", [128, 2 * (b - a)], f32)
            for q, (a, b) in enumerate(zip(CHB[l], CHB[l][1:]))
        ]
        for l in range(3)
    ]
    cc_out = [
        [
            nc.dram_tensor(
                f"cc_out{l}_{q}", [128, 2 * (b - a)], f32, addr_space="Shared"
            )
            for q, (a, b) in enumerate(zip(CHB[l], CHB[l][1:]))
        ]
        for l in range(3)
    ]

    with tile.TileContext(nc) as tc:
        import contextlib

        with contextlib.ExitStack() as ctx:
            # one slot size (4KB/partition) for all activation/weight strips;
            # ring reuse: xt+w0 (16) -> u0 (16) -> u1 (reuses xt/w0) -> u2 (reuses u0)
            act = ctx.enter_context(tc.tile_pool(name="act", bufs=32))
            wpool = ctx.enter_context(tc.tile_pool(name="wstrip", bufs=6))
            pspool = ctx.enter_context(tc.tile_pool(name="psum", bufs=8, space="PSUM"))
            small = ctx.enter_context(tc.tile_pool(name="small", bufs=1))

            # ---- resident loads first; j=0 strips split into 512-col chunks so
            # the first matmul can start ~4us earlier ----
            xt_r = xt_d[:].rearrange("(j p) r -> p j r", p=128)
            w0_r = w0_d[:].rearrange("(j p) f -> p j f", p=128)
            xts, w0s = [], []
            for j in range(KT[0]):
                wt = act.tile([128, D_H], bf16, tag="act", name=f"w0_{j}")
                if j == 0 and full_size:
                    for c in range(4):
                        nc.sync.dma_start(
                            out=wt[:, c * 512 : (c + 1) * 512],
                            in_=w0_r[:, j, c * 512 : (c + 1) * 512],
                        )
                else:
                    nc.sync.dma_start(out=wt, in_=w0_r[:, j, :])
                w0s.append(wt)
                xtile = act.tile([128, R], bf16, tag="act", name=f"xt_{j}")
                if j == 0 and full_size:
                    for c in range(NT):
                        nc.gpsimd.dma_start(
                            out=xtile[:, c * 512 : (c + 1) * 512],
                            in_=xt_r[:, j, c * 512 : (c + 1) * 512],
                        )
                else:
                    nc.gpsimd.dma_start(out=xtile, in_=xt_r[:, j, :])
                xts.append(xtile)

            # ---- constants / per-feature params ----
            eps_t = small.tile([128, 1], f32, tag="eps")
            nc.vector.memset(eps_t, BN_EPS)
            g_t = []
            for l, gd in enumerate((g0_d, g1_d, g2_d)):
                t = small.tile([128, MT[l]], f32, tag=f"g{l}", name=f"g{l}")
                nc.sync.dma_start(out=t, in_=gd[:].rearrange("(m p) -> p m", p=128))
                g_t.append(t)
            b2_t = small.tile([128, MT[2]], f32, tag="b2")
            nc.sync.dma_start(out=b2_t, in_=b2_d[:].rearrange("(m p) -> p m", p=128))

            def u_strips(pool_tag, count, dtype, cols):
                return [
                    act.tile([128, cols], dtype, tag="act", name=f"{pool_tag}_{j}")
                    for j in range(count)
                ]

            def stats_half(l, BN, h, want_c, beta_t, sg_eng=None):
                """bn_stats partials (feature chunk h) -> S/Q -> allreduce -> a [, c].

                cc_in DMA + collective doorbell sit back-to-back on the gpsimd
                queue so every doorbell fires as soon as its local stats land —
                the CC stream then runs ops the moment it frees up. The result
                readback (sg) goes on sg_eng (default gpsimd; L2 passes sync so
                a readback waiting on a slow AllReduce never delays the NEXT
                chunk's doorbell in the gpsimd FIFO).
                """
                m0, m1 = CHB[l][h], CHB[l][h + 1]
                mh = m1 - m0
                mv = small.tile([128, mh, 2], f32, tag=f"mv{l}{h}", name=f"mv{l}{h}")
                for m in range(m0, m0 + mh):
                    nc.vector.bn_aggr(
                        out=mv[:, m - m0, :],
                        in_=BN[:, m * NT * 6 : (m + 1) * NT * 6],
                    )
                # S = mean*R ; Q = (var + mean^2)*R  (exact cross-core sums)
                sf = small.tile([128, 2, mh], f32, tag=f"sf{l}{h}", name=f"sf{l}{h}")
                nc.vector.tensor_scalar_mul(sf[:, 0, :], mv[:, :, 0], float(R))
                nc.vector.tensor_mul(sf[:, 1, :], mv[:, :, 0], mv[:, :, 0])
                nc.vector.tensor_add(sf[:, 1, :], sf[:, 1, :], mv[:, :, 1])
                nc.vector.tensor_scalar_mul(sf[:, 1, :], sf[:, 1, :], float(R))
                nc.gpsimd.dma_start(out=cc_in[l][h][:], in_=sf)
                nc.gpsimd.collective_compute(
                    "AllReduce",
                    Alu.add,
                    replica_groups=GROUP,
                    ins=[cc_in[l][h][:]],
                    outs=[cc_out[l][h][:]],
                )
                sg = small.tile([128, 2, mh], f32, tag=f"sg{l}{h}", name=f"sg{l}{h}")
                (sg_eng or nc.gpsimd).dma_start(
                    out=sg, in_=cc_out[l][h][:].rearrange("p (s m) -> p s m", s=2)
                )
                mean = small.tile([128, mh], f32, tag=f"mean{l}{h}", name=f"mean{l}{h}")
                var = small.tile([128, mh], f32, tag=f"var{l}{h}", name=f"var{l}{h}")
                tmp = small.tile([128, mh], f32, tag=f"tmp{l}{h}", name=f"tmp{l}{h}")
                nc.vector.tensor_scalar_mul(mean, sg[:, 0, :], inv_B)
                nc.vector.tensor_scalar_mul(var, sg[:, 1, :], inv_B)
                nc.vector.tensor_mul(tmp, mean, mean)
                nc.vector.tensor_sub(var, var, tmp)
                # var <- sqrt(var + eps), then reciprocal -> rstd
                nc.scalar.activation(out=var, in_=var, func=Act.Sqrt, bias=eps_t[:, 0:1])
                nc.vector.reciprocal(out=var, in_=var)
                a = small.tile([128, mh], f32, tag=f"a{l}{h}", name=f"a{l}{h}")
                nc.vector.tensor_mul(a, var, g_t[l][:, m0 : m0 + mh])
                if not want_c:
                    return a, None
                c = small.tile([128, mh], f32, tag=f"c{l}{h}", name=f"c{l}{h}")
                nc.vector.tensor_mul(tmp, a, mean)
                nc.vector.tensor_sub(c, beta_t[:, m0 : m0 + mh], tmp)
                return a, c

            SPLIT = 12  # k-tile where the staged strips pause (chunk boundary)

            def layer(l, lhs_getter, rhs_at, dest_at, finish_chunk=None):
                """One linear layer with a staged start and chunked stats.

                Emission order of strips:
                  l=0:        [pair(0,1) j-outer], 2, 3, ..., MT-1
                  l=1,2:      0A,1A,2A,3A, 2B,3B,0B,1B, 4, 5, ..., MT-1
                finish_chunk(q, BN) is emitted as soon as every strip of chunk q
                has its bn_stats emitted.
                """
                BN = small.tile([128, MT[l] * NT * 6], f32, tag=f"BN{l}", name=f"BN{l}")
                done = set()
                ch = [0]

                def strip_stats(m):
                    done.add(m)
                    while (
                        finish_chunk is not None
                        and ch[0] < len(CHB[l]) - 1
                        and all(s in done for s in range(CHB[l][ch[0]], CHB[l][ch[0] + 1]))
                    ):
                        finish_chunk(ch[0], BN)
                        ch[0] += 1

                def psum_copy_stats(m, pss):
                    for n in range(NT):
                        idx = m * NT + n
                        nc.scalar.activation(
                            out=dest_at(m, n), in_=pss[n], func=Act.Copy
                        )
                        nc.vector.bn_stats(
                            out=BN[:, idx * 6 : idx * 6 + 6], in_=pss[n]
                        )
                    strip_stats(m)

                def alloc_ps(m):
                    return [
                        pspool.tile([128, 512], f32, tag="ps", name=f"ps{l}_{m}_{n}")
                        for n in range(NT)
                    ]

                def mm_block(pss, lhs, j, start, stop):
                    for n in range(NT):
                        nc.tensor.matmul(
                            pss[n], lhs(j), rhs_at(j, n), start=start, stop=stop
                        )

                def full_strip(m):
                    lhs = lhs_getter(m)
                    pss = alloc_ps(m)
                    for j in range(KT[l]):
                        mm_block(pss, lhs, j, j == 0, j == KT[l] - 1)
                    psum_copy_stats(m, pss)

                if not full_size:
                    for m in range(MT[l]):
                        full_strip(m)
                    return BN

                if l == 0:
                    # pair-interleave strips 0,1 j-outer: consume each arriving
                    # xt_j/w0_j DMA with two strips' worth of matmuls.
                    lhs0, lhs1 = lhs_getter(0), lhs_getter(1)
                    ps0, ps1 = alloc_ps(0), alloc_ps(1)
                    for j in range(KT[0]):
                        mm_block(ps0, lhs0, j, j == 0, j == KT[0] - 1)
                        mm_block(ps1, lhs1, j, j == 0, j == KT[0] - 1)
                    psum_copy_stats(0, ps0)
                    psum_copy_stats(1, ps1)
                    for m in range(2, MT[l]):
                        full_strip(m)
                    return BN

                # l = 1, 2: staged split. Strips 0..3 run k<SPLIT before anything
                # needs the previous layer's last stat chunk (k>=SPLIT scales).
                lhs = [lhs_getter(m) for m in range(4)]
                ps = {}
                # 0A, 1A: k<SPLIT, close PSUM, park partial (bf16) in dest strips
                for m in (0, 1):
                    ps[m] = alloc_ps(m)
                    for j in range(SPLIT):
                        mm_block(ps[m], lhs[m], j, j == 0, j == SPLIT - 1)
                    for n in range(NT):
                        nc.scalar.activation(
                            out=dest_at(m, n), in_=ps[m][n], func=Act.Copy
                        )
                # 2A, 3A: k<SPLIT, keep PSUM banks open across the split
                for m in (2, 3):
                    ps[m] = alloc_ps(m)
                    for j in range(SPLIT):
                        mm_block(ps[m], lhs[m], j, j == 0, False)
                # 2B, 3B: finish accumulation in the still-open banks
                for m in (2, 3):
                    for j in range(SPLIT, KT[l]):
                        mm_block(ps[m], lhs[m], j, False, j == KT[l] - 1)
                    psum_copy_stats(m, ps[m])
                # 0B, 1B: fresh PSUM group for k>=SPLIT, then DVE add onto the
                # parked partial; bn_stats read the (bf16) summed strip.
                for m in (0, 1):
                    psb = alloc_ps(m)
                    for j in range(SPLIT, KT[l]):
                        mm_block(psb, lhs[m], j, j == SPLIT, j == KT[l] - 1)
                    for n in range(NT):
                        idx = m * NT + n
                        d = dest_at(m, n)
                        nc.vector.tensor_add(d, psb[n], d)
                        nc.vector.bn_stats(out=BN[:, idx * 6 : idx * 6 + 6], in_=d)
                    strip_stats(m)
                for m in range(4, MT[l]):
                    full_strip(m)
                return BN

            def strips_rhs(strips):
                return lambda j, n: strips[j][:, n * 512 : (n + 1) * 512]

            def scale_one(strips, j, ac):
                s = strips[j][:]
                if j % 4 == 3:
                    nc.scalar.activation(out=s, in_=s, func=Act.Copy, scale=ac)
                else:
                    nc.vector.tensor_scalar_mul(s, s, ac)

            def finisher(l, u_next):
                def fin(q, BN):
                    a, _ = stats_half(l, BN, q, False, None)
                    m0 = CHB[l][q]
                    for j in range(m0, CHB[l][q + 1]):
                        scale_one(u_next, j, a[:, j - m0 : j - m0 + 1])
                return fin

            # ================= layer 0 =================
            u0 = u_strips("u0", MT[0], bf16, R)

            def lhs0(m):
                return lambda j: w0s[j][:, m * 128 : (m + 1) * 128]

            layer(0, lhs0, strips_rhs(xts), lambda m, n: strips_rhs(u0)(m, n),
                  finisher(0, u0))

            # ================= layer 1 =================
            u1 = u_strips("u1", MT[1], bf16, R)

            def lhs_strip(w_dram, l):
                def getter(m):
                    w = wpool.tile([128, KT[l] * 128], bf16, tag="w", name=f"w{l}_{m}")
                    nc.sync.dma_start(out=w, in_=w_dram[m])
                    return lambda j: w[:, j * 128 : (j + 1) * 128]

                return getter

            layer(1, lhs_strip(w1_d, 1), strips_rhs(u0), strips_rhs(u1),
                  finisher(1, u1))

            # ================= layer 2 =================
            # u2: bf16 strips, affine in place per chunk, immediate bf16 writeback
            u2 = u_strips("u2", MT[2], bf16, R)

            def fin2(q, BN):
                last_q = q == len(CHB[2]) - 2
                # earlier chunks: sg on sync (never blocks the next doorbell);
                # last chunk: sg on gpsimd (free after the final doorbell, while
                # sync may still be draining a writeback).
                a, c = stats_half(
                    2, BN, q, True, b2_t,
                    sg_eng=nc.gpsimd if last_q else nc.sync,
                )
                m0 = CHB[2][q]
                for m in range(m0, CHB[2][q + 1]):
                    am = a[:, m - m0 : m - m0 + 1]
                    cm = c[:, m - m0 : m - m0 + 1]
                    s = u2[m][:]
                    if m % 2 == 0:
                        nc.vector.tensor_scalar(s, s, am, cm, Alu.mult, Alu.add)
                    else:
                        nc.scalar.activation(
                            out=s, in_=s, func=Act.Identity, bias=cm, scale=am
                        )
                    if last_q and m == CHB[2][q + 1] - 1:
                        # final strip: halve the exposed writeback across queues
                        H = R // 2
                        nc.sync.dma_start(
                            out=out_d[m * 128 : (m + 1) * 128, 0:H], in_=u2[m][:, 0:H]
                        )
                        nc.gpsimd.dma_start(
                            out=out_d[m * 128 : (m + 1) * 128, H:R], in_=u2[m][:, H:R]
                        )
                    else:
                        nc.sync.dma_start(
                            out=out_d[m * 128 : (m + 1) * 128, :], in_=s
                        )

            layer(2, lhs_strip(w2_d, 2), strips_rhs(u1), strips_rhs(u2), fin2)

    nc.compile()
    return nc


def _get_program(R, B_total):
    key = (R, B_total)
    if key not in _PROG_CACHE:
        _PROG_CACHE[key] = build_program(R, B_total)
    return _PROG_CACHE[key]


def prep_inputs(x, W0, W1, W2, gamma0, gamma1, gamma2, beta2, n_cores=N_CORES):
    """Host-side: transpose, cast to bf16, shard batch columns."""
    bf = ml_dtypes.bfloat16

    def strip_tiles(W):
        # W [F, K] -> [F//128 strips, 128 partitions(k%128), (K//128)*128] bf16
        # element [m, p, j*128+f] = W[m*128+f, j*128+p]
        F, Kd = W.shape
        wt = W.T.reshape(Kd // 128, 128, F // 128, 128)  # [j, p, m, f]
        return np.ascontiguousarray(wt.transpose(2, 1, 0, 3)).reshape(
            F // 128, 128, Kd // 128 * 128
        ).astype(bf)

    xT = np.ascontiguousarray(x.T)  # [D_IN, B]
    R = x.shape[0] // n_cores
    w0t = np.ascontiguousarray(W0.T).astype(bf)
    w1t = strip_tiles(np.asarray(W1, dtype=np.float32))
    w2t = strip_tiles(np.asarray(W2, dtype=np.float32))
    g0 = np.ascontiguousarray(gamma0, dtype=np.float32)
    g1 = np.ascontiguousarray(gamma1, dtype=np.float32)
    g2 = np.ascontiguousarray(gamma2, dtype=np.float32)
    b2 = np.ascontiguousarray(beta2, dtype=np.float32)
    in_maps = []
    for c in range(n_cores):
        in_maps.append(
            {
                "xt": np.ascontiguousarray(xT[:, c * R : (c + 1) * R]).astype(bf),
                "w0t": w0t,
                "w1t": w1t,
                "w2t": w2t,
                "g0": g0,
                "g1": g1,
                "g2": g2,
                "beta2": b2,
            }
        )
    return in_maps, R


def kernel(
    x,
    W0,
    b0,
    gamma0,
    beta0,
    W1,
    b1,
    gamma1,
    beta1,
    W2,
    b2,
    gamma2,
    beta2,
):
    """Full-input entry point: shard across 8 NeuronCores, run, gather.

    b0/b1/b2/beta0/beta1 cancel exactly under training-mode BatchNorm
    (shift invariance), so they are not shipped to the device.
    """
    global LAST_RESULTS
    from concourse.bass_utils import run_bass_kernel_spmd

    x = np.asarray(x, dtype=np.float32)
    B = x.shape[0]
    in_maps, R = prep_inputs(
        x, np.asarray(W0), np.asarray(W1), np.asarray(W2),
        np.asarray(gamma0), np.asarray(gamma1), np.asarray(gamma2),
        np.asarray(beta2),
    )
    nc = _get_program(R, B)
    res = run_bass_kernel_spmd(nc, in_maps, core_ids=list(range(N_CORES)))
    LAST_RESULTS = res
    out = np.empty((B, D_OUT), dtype=np.float32)
    for c in range(N_CORES):
        out[c * R : (c + 1) * R, :] = res.results[c]["out"].T.astype(np.float32)
    return out
